# revision 11
# baseline (speedup 1.0000x reference)
"""Trainium2 Bass kernel for CustomPatchEmbedding (ragged patch gather + two projections).

v3 strategy (data-parallel over batch, 8 cores x 4 images):
  - Fine branch (bf16): images repacked on host into a sliding 16-row-block
    channel-last layout; a fine 16x16 patch is ONE contiguous 1536B run and
    one indirect DMA gathers each 128-patch group (the HW DGE consumes exactly
    one offset per destination partition).
  - Coarse branch (fp8 e3m4): a second sliding 64-row-block blob quantized to
    e3m4 on host; a coarse 64x64 patch is ONE contiguous 12288B run and the
    whole coarse gather is a single indirect DMA. Coarse weights are
    pre-scaled by S_W=128, quantized to e3m4, and the output is rescaled by
    1/S_W in the epilogue. Host-measured rel-err of the full pipeline: 0.0069
    (tolerance 2e-2).
  - ALL activation transposes run on the DMA XBAR (dma_start_transpose issued
    from the scalar HWDGE queue), not the PE: gathered [patch, feature] tiles
    become [feature, patch] lhsT tiles directly. The XBAR transposes at 16-bit
    granularity, so fine (bf16) needs no tricks and coarse (fp8) comes out
    pair-interleaved: matmuls use stride-2 lhsT column APs and the coarse
    weight rows are 2-way interleaved on host to match. The PE runs matmuls
    only; no PSUM round-trip or DVE copies.
  - Outputs are written bf16 and upcast to fp32 on host.

kernel(**inputs) takes the FULL unsharded inputs and returns (32, 288, 256) f32.
"""
import sys
import numpy as np

sys.path.insert(0, "/opt/trn_rl_repo")

import ml_dtypes
import concourse.bass as bass
import concourse.bacc as bacc
import concourse.mybir as mybir
import concourse.tile as tile
from concourse.bass_utils import run_bass_kernel_spmd
from contextlib import ExitStack

# Problem constants (hardcoded per spec).
B, C, H, W = 32, 3, 512, 512
FP, CP = 16, 64
NF, NCO = 256, 32
D = 256
NCORES = 8
IPC = B // NCORES              # images per core
KF = C * FP * FP               # 768   fine features
KC = C * CP * CP               # 12288 coarse features
P = 128
GF = IPC * 2                   # 8 fine groups of 128 patches per core
S_W = 128.0                    # coarse-weight pre-scale before e3m4 quantization

RUN_F = FP * FP * C            # 768 elements per fine gather run (whole patch)
BLK_F = W * FP * C             # fine blob stride per y-block
NROW_F = H - FP + 1            # 497 y-blocks stored
IMG_F = NROW_F * BLK_F
BLK_C = W * CP * C             # coarse blob stride per y-block
NROW_C = H - CP + 1            # 449 y-blocks stored
IMG_C = NROW_C * BLK_C

NJF = KF // P                  # 6 fine k-chunks
NSC = KC // (2 * P)            # 48 coarse u16-transpose blocks (2 fp8 chunks each)
NTC = 4                        # coarse XBAR transposes (12 blocks each)
SPT = NSC // NTC               # 12 blocks per coarse XBAR

FDT = mybir.dt.float32
BDT = mybir.dt.bfloat16
F8 = mybir.dt.float8e3
U16 = mybir.dt.uint16
IDT = mybir.dt.int32
BF16 = ml_dtypes.bfloat16
E3M4 = ml_dtypes.float8_e3m4


def _emit(nc, tc, t):
    """Emit the per-core Tile program. `t` maps tensor name -> dram handle."""
    with ExitStack() as ctx:
        const = ctx.enter_context(tc.tile_pool(name="const", bufs=1))
        gf_pool = ctx.enter_context(tc.tile_pool(name="gf", bufs=GF))
        wc_pool = ctx.enter_context(tc.tile_pool(name="wc", bufs=4))
        ltf_pool = ctx.enter_context(tc.tile_pool(name="ltf", bufs=3))
        ltc_pool = ctx.enter_context(tc.tile_pool(name="ltc", bufs=NTC))
        ob_pool = ctx.enter_context(tc.tile_pool(name="ob", bufs=3))
        ps_f = ctx.enter_context(tc.tile_pool(name="ps_f", bufs=2, space="PSUM"))
        ps_c = ctx.enter_context(tc.tile_pool(name="ps_c", bufs=1, space="PSUM"))

        # --- offsets first so gathers can start immediately ---
        cidx = const.tile([P, 1], IDT)
        nc.sync.dma_start(cidx[:], t["cidx"][:])
        fidx = const.tile([P, GF], IDT)
        nc.sync.dma_start(fidx[:], t["fidx"][:])
        bias_f = const.tile([P, D], FDT)
        nc.sync.dma_start(bias_f[:], t["bias_f"][:])
        bias_c = const.tile([P, D], FDT)
        nc.sync.dma_start(bias_c[:], t["bias_c"][:])
        wf = const.tile([P, NJF * D], BDT)
        nc.sync.dma_start(wf[:], t["wf2"][:])
        wc = []
        for s in range(4):
            wt = wc_pool.tile([P, 24 * D], F8, tag="wc")
            nc.sync.dma_start(wt[:], t["wc2"][:, s * 24 * D:(s + 1) * 24 * D])
            wc.append(wt)

        # --- gathers: coarse first (it feeds the long XBAR+matmul chain),
        # then the 8 fine groups. One offset per destination partition.
        gc = const.tile([P, KC], F8)
        nc.gpsimd.indirect_dma_start(
            out=gc[:], out_offset=None, in_=t["imgs8c"][:],
            in_offset=bass.IndirectOffsetOnAxis(ap=cidx[:, 0:1], axis=0),
        )
        gfs = []
        for g in range(GF):
            gt = gf_pool.tile([P, RUN_F], BDT, tag="gf")
            gfs.append(gt)
            nc.gpsimd.indirect_dma_start(
                out=gt[:], out_offset=None, in_=t["imgs16"][:],
                in_offset=bass.IndirectOffsetOnAxis(ap=fidx[:, g:g + 1], axis=0),
            )

        out = t["out"]
        psum_c = ps_c.tile([P, D], FDT)
        gc16 = gc[:].bitcast(U16)                      # [128, 6144]

        # --- stages ---
        def fine_T(g):
            lt = ltf_pool.tile([P, NJF, P], BDT, tag="ltf")
            nc.scalar.dma_start_transpose(lt[:], gfs[g][:])
            return lt

        def fine_M(g, lt):
            psum = ps_f.tile([P, D], FDT, tag="psf")
            for j in range(NJF):
                nc.tensor.matmul(
                    out=psum[:], lhsT=lt[:, j, :],
                    rhs=wf[:, j * D:(j + 1) * D],
                    start=(j == 0), stop=(j == NJF - 1),
                )
            ob = ob_pool.tile([P, D], BDT, tag="ob")
            nc.vector.tensor_tensor(
                out=ob[:], in0=psum[:], in1=bias_f[:], op=mybir.AluOpType.add
            )
            b_img, hh = divmod(g, 2)
            row0 = b_img * (NF + NCO) + hh * P
            nc.sync.dma_start(out[row0:row0 + P, :], ob[:])

        def coarse_T(q):
            lt = ltc_pool.tile([P, SPT, P], U16, tag="ltc")
            nc.scalar.dma_start_transpose(
                lt[:], gc16[:, q * SPT * P:(q + 1) * SPT * P])
            return lt

        def coarse_M(q, lt):
            for s_loc in range(SPT):
                blk8 = lt[:, s_loc, :].bitcast(F8)     # [128, 256]
                s = q * SPT + s_loc
                for b_par in range(2):
                    blk = 2 * s + b_par
                    nc.tensor.matmul(
                        out=psum_c[:],
                        lhsT=blk8[:, b_par:2 * P:2],
                        rhs=wc[blk // 24][:, (blk % 24) * D:(blk % 24 + 1) * D],
                        start=(blk == 0), stop=(blk == 2 * NSC - 1),
                    )
            if q == NTC - 1:
                oc = ob_pool.tile([P, D], BDT, tag="ob")
                nc.vector.scalar_tensor_tensor(
                    out=oc[:], in0=psum_c[:], scalar=1.0 / S_W, in1=bias_c[:],
                    op0=mybir.AluOpType.mult, op1=mybir.AluOpType.add,
                )
                out3 = out[:].rearrange("(b r) d -> b r d", b=IPC)
                nc.scalar.dma_start(out3[:, NF:NF + NCO, :], oc[:])

        # --- schedule: fine groups stream through PE while the coarse XBAR
        # transposes run on the scalar DMA queue; coarse matmuls follow.
        lts = {}
        lts["f0"] = fine_T(0)
        lts["f1"] = fine_T(1)
        for g in range(2, GF):
            fine_M(g - 2, lts[f"f{g-2}"])
            lts[f"f{g}"] = fine_T(g)
            if g >= 4:
                lts[f"c{g-4}"] = coarse_T(g - 4)
        fine_M(GF - 2, lts[f"f{GF-2}"])
        fine_M(GF - 1, lts[f"f{GF-1}"])
        for q in range(NTC):
            coarse_M(q, lts[f"c{q}"])


def build(reps: int = 1):
    nc = bacc.Bacc("TRN2", target_bir_lowering=False, debug=False)
    t = {
        "imgs16": nc.dram_tensor("imgs16", [IPC * IMG_F, 1], BDT, kind="ExternalInput"),
        "imgs8c": nc.dram_tensor("imgs8c", [IPC * IMG_C, 1], F8, kind="ExternalInput"),
        "wf2": nc.dram_tensor("wf2", [P, NJF * D], BDT, kind="ExternalInput"),
        "wc2": nc.dram_tensor("wc2", [P, (KC // P) * D], F8, kind="ExternalInput"),
        "bias_f": nc.dram_tensor("bias_f", [P, D], FDT, kind="ExternalInput"),
        "bias_c": nc.dram_tensor("bias_c", [P, D], FDT, kind="ExternalInput"),
        "fidx": nc.dram_tensor("fidx", [P, GF], IDT, kind="ExternalInput"),
        "cidx": nc.dram_tensor("cidx", [P, 1], IDT, kind="ExternalInput"),
        "out": nc.dram_tensor("out", [IPC * (NF + NCO), D], BDT, kind="ExternalOutput"),
    }
    with tile.TileContext(nc) as tc:
        for _ in range(reps):
            _emit(nc, tc, t)
    nc.compile()
    return nc


def repack_fine(images):
    """[b, C, H, W] f32 -> sliding 16-row-block channel-last bf16 blob.

    blk[b, y, x, dy, c] = images[b, c, y+dy, x], y in [0, H-16]."""
    cl = np.ascontiguousarray(images.transpose(0, 2, 3, 1)).astype(BF16)
    sw = np.lib.stride_tricks.sliding_window_view(cl, FP, axis=1)  # [b,497,x,c,dy]
    return np.ascontiguousarray(sw.transpose(0, 1, 2, 4, 3))


def repack_coarse(images):
    """[b, C, H, W] f32 -> sliding 64-row-block channel-last e3m4 blob."""
    cl = np.ascontiguousarray(images.transpose(0, 2, 3, 1)).astype(E3M4)
    sw = np.lib.stride_tricks.sliding_window_view(cl, CP, axis=1)  # [b,449,x,c,dy]
    return np.ascontiguousarray(sw.transpose(0, 1, 2, 4, 3))


def host_indices(fine_xy, coarse_xy):
    """Element offsets into the per-core blobs (one per gather run)."""
    base_f = fine_xy[:, :, 1] * BLK_F + fine_xy[:, :, 0] * (FP * C) \
        + (np.arange(IPC) * IMG_F)[:, None]                        # [IPC, NF]
    fidx = base_f.reshape(GF, P).T                                 # [P, GF]
    base_c = coarse_xy[:, :, 1] * BLK_C + coarse_xy[:, :, 0] * (CP * C) \
        + (np.arange(IPC) * IMG_C)[:, None]                        # [IPC, NCO]
    cidx = base_c.reshape(P, 1)
    return (np.ascontiguousarray(fidx.astype(np.int32)),
            np.ascontiguousarray(cidx.astype(np.int32)))


def feat_perm(patch):
    """Gather order (dx, dy, c) -> original (c, dy, dx) column index."""
    dx, dy, c = np.meshgrid(
        np.arange(patch), np.arange(patch), np.arange(C), indexing="ij"
    )
    return (c * (patch * patch) + dy * patch + dx).reshape(-1)


def swizzle_w_interleave(wg, stride):
    """[K, D] gather-order weights -> [128, (K//128)*D], rows interleaved so
    block (j, b) holds rows (128*stride)*j + stride*i + b (i = partition)."""
    K = wg.shape[0]
    blocks = []
    for j in range(K // (P * stride)):
        for b in range(stride):
            blocks.append(wg[P * stride * j + stride * np.arange(P) + b])
    return np.ascontiguousarray(
        np.stack(blocks, axis=1).reshape(P, (K // P) * D)
    )


def make_in_maps(images, W_fine, b_fine, W_coarse, b_coarse, fine_xy, coarse_xy):
    images = np.asarray(images, dtype=np.float32)
    fine_xy = np.asarray(fine_xy, dtype=np.int64)
    coarse_xy = np.asarray(coarse_xy, dtype=np.int64)
    blob_f = repack_fine(images)
    blob_c = repack_coarse(images)
    wf2 = swizzle_w_interleave(
        np.asarray(W_fine, np.float32).T[feat_perm(FP)].astype(BF16), 1)
    wc2 = swizzle_w_interleave(
        (np.asarray(W_coarse, np.float32).T[feat_perm(CP)] * S_W).astype(E3M4), 2)
    bias_f = np.ascontiguousarray(
        np.repeat(np.asarray(b_fine, np.float32)[None, :], P, axis=0))
    bias_c = np.ascontiguousarray(
        np.repeat(np.asarray(b_coarse, np.float32)[None, :], P, axis=0))
    in_maps = []
    for c in range(NCORES):
        sl = slice(c * IPC, (c + 1) * IPC)
        fidx, cidx = host_indices(fine_xy[sl], coarse_xy[sl])
        in_maps.append({
            "imgs16": blob_f[sl].reshape(IPC * IMG_F, 1),
            "imgs8c": blob_c[sl].reshape(IPC * IMG_C, 1),
            "wf2": wf2, "wc2": wc2,
            "bias_f": bias_f, "bias_c": bias_c,
            "fidx": fidx, "cidx": cidx,
        })
    return in_maps


_NC_CACHE = []


def _get_nc():
    if not _NC_CACHE:
        _NC_CACHE.append(build())
    return _NC_CACHE[0]


def run(inputs: dict, trace: bool = False):
    nc = _get_nc()
    in_maps = make_in_maps(**inputs)
    res = run_bass_kernel_spmd(nc, in_maps, list(range(NCORES)), trace=trace)
    outs = [
        np.asarray(res.results[c]["out"]).astype(np.float32).reshape(IPC, NF + NCO, D)
        for c in range(NCORES)
    ]
    return np.concatenate(outs, axis=0), res


def kernel(**inputs) -> np.ndarray:
    out, _ = run(inputs, trace=False)
    return out


# revision 13
# speedup vs baseline: 1.5002x; 1.5002x over previous
"""Trainium2 Bass kernel for CustomPatchEmbedding (ragged patch gather + two projections).

v4 strategy (data-parallel over batch, 8 cores x 4 images):
  - Fine branch (bf16): images repacked on host into a sliding 16-row-block
    channel-last layout; a fine 16x16 patch is ONE contiguous 1536B run and
    one indirect DMA gathers each 128-patch group (the HW DGE consumes exactly
    one offset per destination partition; multi-offset APs are silently
    truncated, verified on HW).
  - Coarse branch (fp8 e3m4): a second sliding 64-row-block blob quantized to
    e3m4 on host; a coarse 64x64 patch is ONE contiguous 12288B run and the
    whole coarse gather is a single indirect DMA. Coarse weights are
    pre-scaled by S_W=128 and quantized to e3m4 (halves weight traffic); the
    epilogue rescales by 1/S_W. Host-measured rel-err: 0.0069 (tol 2e-2).
  - Activation transposes run on the PE, packed through fp32r views: one
    [128,128] fp32r transpose moves 2 bf16 (fine) or 4 fp8 (coarse) k-chunks;
    matmuls then use stride-2/stride-4 lhsT column APs, with weight rows
    interleaved on host to match. (The DMA XBAR transpose was tried and is
    poison on HW: it emits ~250B packets that clog the shared DMA engines.)
  - DMA scheduling: the fine gathers are the latency-critical stream, so the
    coarse gather is issued after them and the big coarse-weight loads are
    issued on the GpSimd software queue BEHIND the gathers — queue-0 FIFO
    order effectively deprioritizes weight traffic without starving it.
  - Outputs are written bf16 (upcast to fp32 on host); the coarse epilogue is
    a single scalar_tensor_tensor and a single 3D-AP output DMA.

kernel(**inputs) takes the FULL unsharded inputs and returns (32, 288, 256) f32.
"""
import sys
import numpy as np

sys.path.insert(0, "/opt/trn_rl_repo")

import ml_dtypes
import concourse.bass as bass
import concourse.bacc as bacc
import concourse.mybir as mybir
import concourse.tile as tile
from concourse.bass_utils import run_bass_kernel_spmd
from contextlib import ExitStack

# Problem constants (hardcoded per spec).
B, C, H, W = 32, 3, 512, 512
FP, CP = 16, 64
NF, NCO = 256, 32
D = 256
NCORES = 8
IPC = B // NCORES              # images per core
KF = C * FP * FP               # 768   fine features
KC = C * CP * CP               # 12288 coarse features
P = 128
GF = IPC * 2                   # 8 fine groups of 128 patches per core
S_W = 128.0                    # coarse-weight pre-scale before e3m4 quantization

RUN_F = FP * FP * C            # 768 elements per fine gather run (whole patch)
BLK_F = W * FP * C             # fine blob stride per y-block
NROW_F = H - FP + 1            # 497 y-blocks stored
IMG_F = NROW_F * BLK_F
BLK_C = W * CP * C             # coarse blob stride per y-block
NROW_C = H - CP + 1            # 449 y-blocks stored
IMG_C = NROW_C * BLK_C

NQF = KF // (2 * P)            # 3 fine fp32-transpose blocks (2 bf16 chunks each)
NQC = KC // (4 * P)            # 24 coarse fp32-transpose blocks (4 fp8 chunks each)
NTC = NQC // 3                 # 8 coarse transpose tiles (3 blocks per tile)

FDT = mybir.dt.float32
RDT = mybir.dt.float32    # float32r transposes are 1.5 cyc/row vs 2.0 but fail BIR verification
BDT = mybir.dt.bfloat16
F8 = mybir.dt.float8e3
IDT = mybir.dt.int32
BF16 = ml_dtypes.bfloat16
E3M4 = ml_dtypes.float8_e3m4


def _emit(nc, tc, t):
    """Emit the per-core Tile program. `t` maps tensor name -> dram handle."""
    with ExitStack() as ctx:
        const = ctx.enter_context(tc.tile_pool(name="const", bufs=1))
        gf_pool = ctx.enter_context(tc.tile_pool(name="gf", bufs=GF))
        wc_pool = ctx.enter_context(tc.tile_pool(name="wc", bufs=4))
        lt_f = ctx.enter_context(tc.tile_pool(name="lt_f", bufs=3))
        lt_c = ctx.enter_context(tc.tile_pool(name="lt_c", bufs=3))
        ob_pool = ctx.enter_context(tc.tile_pool(name="ob", bufs=3))
        ps_tp = ctx.enter_context(tc.tile_pool(name="ps_tp", bufs=3, space="PSUM"))
        ps_f = ctx.enter_context(tc.tile_pool(name="ps_f", bufs=2, space="PSUM"))
        ps_c = ctx.enter_context(tc.tile_pool(name="ps_c", bufs=1, space="PSUM"))

        # --- offsets first so gathers can start immediately ---
        fidx = const.tile([P, GF], IDT)
        nc.sync.dma_start(fidx[:], t["fidx"][:])
        cidx = const.tile([P, 1], IDT)
        nc.sync.dma_start(cidx[:], t["cidx"][:])
        ident = const.tile([P, P], RDT)
        nc.sync.dma_start(ident[:], t["ident"][:])
        wf = const.tile([P, (KF // P) * D], BDT)
        nc.sync.dma_start(wf[:], t["wf2"][:])
        bias_f = const.tile([P, D], FDT)
        nc.sync.dma_start(bias_f[:], t["bias_f"][:])
        bias_c = const.tile([P, D], FDT)
        nc.sync.dma_start(bias_c[:], t["bias_c"][:])

        # --- gathers: the 8 fine groups are the latency-critical stream and
        # go first; the coarse gather follows; the coarse weight loads are
        # issued on the same software queue AFTER them (FIFO deprioritization).
        gfs = []
        for g in range(GF):
            gt = gf_pool.tile([P, RUN_F], BDT, tag="gf")
            gfs.append(gt)
            nc.gpsimd.indirect_dma_start(
                out=gt[:], out_offset=None, in_=t["imgs16"][:],
                in_offset=bass.IndirectOffsetOnAxis(ap=fidx[:, g:g + 1], axis=0),
            )
        gc = const.tile([P, KC], F8)
        nc.gpsimd.indirect_dma_start(
            out=gc[:], out_offset=None, in_=t["imgs8c"][:],
            in_offset=bass.IndirectOffsetOnAxis(ap=cidx[:, 0:1], axis=0),
        )
        wc = []
        for s in range(4):
            wt = wc_pool.tile([P, 24 * D], F8, tag="wc")
            nc.gpsimd.dma_start(wt[:], t["wc2"][:, s * 24 * D:(s + 1) * 24 * D])
            wc.append(wt)

        out = t["out"]
        psum_c = ps_c.tile([P, D], FDT)

        # --- stages: T (packed PE transposes + DVE copy), M (matmuls) ---
        def fine_T(g):
            g32 = gfs[g][:].bitcast(RDT)               # [128, 384]
            tp = ps_tp.tile([P, NQF * P], RDT, tag="tp")
            for j in range(NQF):
                nc.tensor.transpose(
                    out=tp[:, j * P:(j + 1) * P],
                    in_=g32[:, j * P:(j + 1) * P],
                    identity=ident[:],
                )
            lt = lt_f.tile([P, NQF * P], RDT, tag="ltf")
            nc.vector.tensor_copy(lt[:], tp[:])
            return lt

        def fine_M(g, lt):
            psum = ps_f.tile([P, D], FDT, tag="psf")
            ltb = lt[:].bitcast(BDT)                   # [128, 768]
            for j in range(NQF):
                for b_par in range(2):
                    blk = 2 * j + b_par
                    nc.tensor.matmul(
                        out=psum[:],
                        lhsT=ltb[:, 2 * P * j + b_par:2 * P * (j + 1):2],
                        rhs=wf[:, blk * D:(blk + 1) * D],
                        start=(blk == 0), stop=(blk == 2 * NQF - 1),
                    )
            ob = ob_pool.tile([P, D], BDT, tag="ob")
            nc.vector.tensor_tensor(
                out=ob[:], in0=psum[:], in1=bias_f[:], op=mybir.AluOpType.add
            )
            b_img, hh = divmod(g, 2)
            row0 = b_img * (NF + NCO) + hh * P
            nc.scalar.dma_start(out[row0:row0 + P, :], ob[:])

        def coarse_T(tt):
            gc32 = gc[:].bitcast(RDT)                  # [128, 3072]
            tp = ps_tp.tile([P, 3 * P], RDT, tag="tp")
            for q in range(3):
                j = 3 * tt + q
                nc.tensor.transpose(
                    out=tp[:, q * P:(q + 1) * P],
                    in_=gc32[:, j * P:(j + 1) * P],
                    identity=ident[:],
                )
            lt = lt_c.tile([P, 3 * P], RDT, tag="ltc")
            nc.vector.tensor_copy(lt[:], tp[:])
            return lt

        def coarse_M(tt, lt):
            lt8 = lt[:].bitcast(F8)                    # [128, 1536]
            for q in range(3):
                j = 3 * tt + q
                for b_par in range(4):
                    blk = 4 * j + b_par
                    nc.tensor.matmul(
                        out=psum_c[:],
                        lhsT=lt8[:, 4 * P * q + b_par:4 * P * (q + 1):4],
                        rhs=wc[blk // 24][:, (blk % 24) * D:(blk % 24 + 1) * D],
                        start=(blk == 0), stop=(blk == 4 * NQC - 1),
                    )
            if tt == NTC - 1:
                oc = ob_pool.tile([P, D], BDT, tag="ob")
                nc.vector.scalar_tensor_tensor(
                    out=oc[:], in0=psum_c[:], scalar=1.0 / S_W, in1=bias_c[:],
                    op0=mybir.AluOpType.mult, op1=mybir.AluOpType.add,
                )
                out3 = out[:].rearrange("(b r) d -> b r d", b=IPC)
                nc.scalar.dma_start(out3[:, NF:NF + NCO, :], oc[:])

        # --- emit with 1-stage software pipelining: T(s+1) before M(s) ---
        stages = [("f", g) for g in range(GF)] + [("c", tt) for tt in range(NTC)]
        prev = None
        for kind, i in stages:
            lt = fine_T(i) if kind == "f" else coarse_T(i)
            if prev is not None:
                pk, pi, plt = prev
                (fine_M if pk == "f" else coarse_M)(pi, plt)
            prev = (kind, i, lt)
        pk, pi, plt = prev
        (fine_M if pk == "f" else coarse_M)(pi, plt)


def build(reps: int = 1):
    nc = bacc.Bacc("TRN2", target_bir_lowering=False, debug=False)
    t = {
        "imgs16": nc.dram_tensor("imgs16", [IPC * IMG_F, 1], BDT, kind="ExternalInput"),
        "imgs8c": nc.dram_tensor("imgs8c", [IPC * IMG_C, 1], F8, kind="ExternalInput"),
        "wf2": nc.dram_tensor("wf2", [P, (KF // P) * D], BDT, kind="ExternalInput"),
        "wc2": nc.dram_tensor("wc2", [P, (KC // P) * D], F8, kind="ExternalInput"),
        "bias_f": nc.dram_tensor("bias_f", [P, D], FDT, kind="ExternalInput"),
        "bias_c": nc.dram_tensor("bias_c", [P, D], FDT, kind="ExternalInput"),
        "ident": nc.dram_tensor("ident", [P, P], RDT, kind="ExternalInput"),
        "fidx": nc.dram_tensor("fidx", [P, GF], IDT, kind="ExternalInput"),
        "cidx": nc.dram_tensor("cidx", [P, 1], IDT, kind="ExternalInput"),
        "out": nc.dram_tensor("out", [IPC * (NF + NCO), D], BDT, kind="ExternalOutput"),
    }
    with tile.TileContext(nc) as tc:
        for _ in range(reps):
            _emit(nc, tc, t)
    nc.compile()
    return nc


def repack_fine(images):
    """[b, C, H, W] f32 -> sliding 16-row-block channel-last bf16 blob.

    blk[b, y, x, dy, c] = images[b, c, y+dy, x], y in [0, H-16]."""
    cl = np.ascontiguousarray(images.transpose(0, 2, 3, 1)).astype(BF16)
    sw = np.lib.stride_tricks.sliding_window_view(cl, FP, axis=1)  # [b,497,x,c,dy]
    return np.ascontiguousarray(sw.transpose(0, 1, 2, 4, 3))


def repack_coarse(images):
    """[b, C, H, W] f32 -> sliding 64-row-block channel-last e3m4 blob."""
    cl = np.ascontiguousarray(images.transpose(0, 2, 3, 1)).astype(E3M4)
    sw = np.lib.stride_tricks.sliding_window_view(cl, CP, axis=1)  # [b,449,x,c,dy]
    return np.ascontiguousarray(sw.transpose(0, 1, 2, 4, 3))


def host_indices(fine_xy, coarse_xy):
    """Element offsets into the per-core blobs (one per gather run)."""
    base_f = fine_xy[:, :, 1] * BLK_F + fine_xy[:, :, 0] * (FP * C) \
        + (np.arange(IPC) * IMG_F)[:, None]                        # [IPC, NF]
    fidx = base_f.reshape(GF, P).T                                 # [P, GF]
    base_c = coarse_xy[:, :, 1] * BLK_C + coarse_xy[:, :, 0] * (CP * C) \
        + (np.arange(IPC) * IMG_C)[:, None]                        # [IPC, NCO]
    cidx = base_c.reshape(P, 1)
    return (np.ascontiguousarray(fidx.astype(np.int32)),
            np.ascontiguousarray(cidx.astype(np.int32)))


def feat_perm(patch):
    """Gather order (dx, dy, c) -> original (c, dy, dx) column index."""
    dx, dy, c = np.meshgrid(
        np.arange(patch), np.arange(patch), np.arange(C), indexing="ij"
    )
    return (c * (patch * patch) + dy * patch + dx).reshape(-1)


def swizzle_w_interleave(wg, stride):
    """[K, D] gather-order weights -> [128, (K//128)*D], rows interleaved so
    block (j, b) holds rows (128*stride)*j + stride*i + b (i = partition)."""
    K = wg.shape[0]
    blocks = []
    for j in range(K // (P * stride)):
        for b in range(stride):
            blocks.append(wg[P * stride * j + stride * np.arange(P) + b])
    return np.ascontiguousarray(
        np.stack(blocks, axis=1).reshape(P, (K // P) * D)
    )


def make_in_maps(images, W_fine, b_fine, W_coarse, b_coarse, fine_xy, coarse_xy):
    images = np.asarray(images, dtype=np.float32)
    fine_xy = np.asarray(fine_xy, dtype=np.int64)
    coarse_xy = np.asarray(coarse_xy, dtype=np.int64)
    blob_f = repack_fine(images)
    blob_c = repack_coarse(images)
    wf2 = swizzle_w_interleave(
        np.asarray(W_fine, np.float32).T[feat_perm(FP)].astype(BF16), 2)
    wc2 = swizzle_w_interleave(
        (np.asarray(W_coarse, np.float32).T[feat_perm(CP)] * S_W).astype(E3M4), 4)
    bias_f = np.ascontiguousarray(
        np.repeat(np.asarray(b_fine, np.float32)[None, :], P, axis=0))
    bias_c = np.ascontiguousarray(
        np.repeat(np.asarray(b_coarse, np.float32)[None, :], P, axis=0))
    ident = np.eye(P, dtype=np.float32)
    in_maps = []
    for c in range(NCORES):
        sl = slice(c * IPC, (c + 1) * IPC)
        fidx, cidx = host_indices(fine_xy[sl], coarse_xy[sl])
        in_maps.append({
            "imgs16": blob_f[sl].reshape(IPC * IMG_F, 1),
            "imgs8c": blob_c[sl].reshape(IPC * IMG_C, 1),
            "wf2": wf2, "wc2": wc2,
            "bias_f": bias_f, "bias_c": bias_c, "ident": ident,
            "fidx": fidx, "cidx": cidx,
        })
    return in_maps


_NC_CACHE = []


def _get_nc():
    if not _NC_CACHE:
        _NC_CACHE.append(build())
    return _NC_CACHE[0]


def run(inputs: dict, trace: bool = False):
    nc = _get_nc()
    in_maps = make_in_maps(**inputs)
    res = run_bass_kernel_spmd(nc, in_maps, list(range(NCORES)), trace=trace)
    outs = [
        np.asarray(res.results[c]["out"]).astype(np.float32).reshape(IPC, NF + NCO, D)
        for c in range(NCORES)
    ]
    return np.concatenate(outs, axis=0), res


def kernel(**inputs) -> np.ndarray:
    out, _ = run(inputs, trace=False)
    return out


# revision 22
# speedup vs baseline: 1.6707x; 1.1137x over previous
"""Trainium2 Bass kernel for CustomPatchEmbedding (ragged patch gather + two projections).

v4 strategy (data-parallel over batch, 8 cores x 4 images):
  - Fine branch (bf16): images repacked on host into a sliding 16-row-block
    channel-last layout; a fine 16x16 patch is ONE contiguous 1536B run and
    one indirect DMA gathers each 128-patch group (the HW DGE consumes exactly
    one offset per destination partition; multi-offset APs are silently
    truncated, verified on HW).
  - Coarse branch (fp8 e3m4): a second sliding 64-row-block blob quantized to
    e3m4 on host; a coarse 64x64 patch is ONE contiguous 12288B run and the
    whole coarse gather is a single indirect DMA. Coarse weights are
    pre-scaled by S_W=128 and quantized to e3m4 (halves weight traffic); the
    epilogue rescales by 1/S_W. Host-measured rel-err: 0.0069 (tol 2e-2).
  - Activation transposes run on the PE, packed through fp32r views: one
    [128,128] fp32r transpose moves 2 bf16 (fine) or 4 fp8 (coarse) k-chunks;
    matmuls then use stride-2/stride-4 lhsT column APs, with weight rows
    interleaved on host to match. (The DMA XBAR transpose was tried and is
    poison on HW: it emits ~250B packets that clog the shared DMA engines.)
  - DMA scheduling: the fine gathers are the latency-critical stream, so the
    coarse gather is issued after them and the big coarse-weight loads are
    issued on the GpSimd software queue BEHIND the gathers — queue-0 FIFO
    order effectively deprioritizes weight traffic without starving it.
  - Outputs are written bf16 (upcast to fp32 on host); the coarse epilogue is
    a single scalar_tensor_tensor and a single 3D-AP output DMA.

kernel(**inputs) takes the FULL unsharded inputs and returns (32, 288, 256) f32.
"""
import sys
import numpy as np

sys.path.insert(0, "/opt/trn_rl_repo")

import ml_dtypes
import concourse.bass as bass
import concourse.bacc as bacc
import concourse.mybir as mybir
import concourse.tile as tile
from concourse.bass_utils import run_bass_kernel_spmd
from contextlib import ExitStack

# Problem constants (hardcoded per spec).
B, C, H, W = 32, 3, 512, 512
FP, CP = 16, 64
NF, NCO = 256, 32
D = 256
NCORES = 8
IPC = B // NCORES              # images per core
KF = C * FP * FP               # 768   fine features
KC = C * CP * CP               # 12288 coarse features
P = 128
GF = IPC * 2                   # 8 fine groups of 128 patches per core
S_W = 128.0                    # coarse-weight pre-scale before e3m4 quantization

RUN_F = FP * FP * C            # 768 elements per fine gather run (whole patch)
BLK_F = W * FP * C             # fine blob stride per y-block
NROW_F = H - FP + 1            # 497 y-blocks stored
IMG_F = NROW_F * BLK_F
BLK_C = W * CP * C             # coarse blob stride per y-block
NROW_C = H - CP + 1            # 449 y-blocks stored
IMG_C = NROW_C * BLK_C

NQF = KF // (2 * P)            # 3 fine fp32-transpose blocks (2 bf16 chunks each)
NQC = KC // (4 * P)            # 24 coarse fp32-transpose blocks (4 fp8 chunks each)
NTC = NQC // 3                 # 8 coarse transpose tiles (3 blocks per tile)

FDT = mybir.dt.float32
RDT = mybir.dt.float32    # float32r transposes are 1.5 cyc/row vs 2.0 but fail BIR verification
BDT = mybir.dt.bfloat16
F8 = mybir.dt.float8e3
IDT = mybir.dt.int32
BF16 = ml_dtypes.bfloat16
E3M4 = ml_dtypes.float8_e3m4


def _emit(nc, tc, t):
    """Emit the per-core Tile program. `t` maps tensor name -> dram handle."""
    with ExitStack() as ctx:
        const = ctx.enter_context(tc.tile_pool(name="const", bufs=1))
        gf_pool = ctx.enter_context(tc.tile_pool(name="gf", bufs=GF))
        wc_pool = ctx.enter_context(tc.tile_pool(name="wc", bufs=4))
        lt_f = ctx.enter_context(tc.tile_pool(name="lt_f", bufs=3))
        lt_c = ctx.enter_context(tc.tile_pool(name="lt_c", bufs=3))
        ob_pool = ctx.enter_context(tc.tile_pool(name="ob", bufs=3))
        ps_tp = ctx.enter_context(tc.tile_pool(name="ps_tp", bufs=3, space="PSUM"))
        ps_f = ctx.enter_context(tc.tile_pool(name="ps_f", bufs=2, space="PSUM"))
        ps_c = ctx.enter_context(tc.tile_pool(name="ps_c", bufs=1, space="PSUM"))

        # --- offsets first so gathers can start immediately ---
        fidx = const.tile([P, GF], IDT)
        nc.sync.dma_start(fidx[:], t["fidx"][:])
        cidx = const.tile([P, 1], IDT)
        nc.sync.dma_start(cidx[:], t["cidx"][:])
        ident = const.tile([P, P], RDT)
        nc.sync.dma_start(ident[:], t["ident"][:])
        identb = const.tile([P, P], BDT)
        nc.sync.dma_start(identb[:], t["identb"][:])
        wf = const.tile([P, (KF // P) * D], BDT)
        nc.sync.dma_start(wf[:], t["wf2"][:])
        bias_f = const.tile([P, D], FDT)
        nc.sync.dma_start(bias_f[:], t["bias_f"][:])
        bias_c = const.tile([P, D], FDT)
        nc.sync.dma_start(bias_c[:], t["bias_c"][:])

        # --- gathers: the 8 fine groups are the latency-critical stream and
        # go first; the coarse gather follows; the coarse weight loads are
        # issued on the same software queue AFTER them (FIFO deprioritization).
        gfs = []
        gc = const.tile([P, KC], F8)

        def emit_gather_f(g):
            nc.gpsimd.indirect_dma_start(
                out=gfs[g][:], out_offset=None, in_=t["imgs16"][:],
                in_offset=bass.IndirectOffsetOnAxis(ap=fidx[:, g:g + 1], axis=0),
            )

        for g in range(GF):
            gfs.append(gf_pool.tile([P, RUN_F], BDT, tag="gf", name=f"gf{g}"))
        for g in range(4):
            emit_gather_f(g)
        nc.gpsimd.indirect_dma_start(
            out=gc[:], out_offset=None, in_=t["imgs8c"][:],
            in_offset=bass.IndirectOffsetOnAxis(ap=cidx[:, 0:1], axis=0),
        )
        for g in range(4, GF):
            emit_gather_f(g)
        wc = []
        for s in range(4):
            wt = wc_pool.tile([P, 24 * D], F8, tag="wc")
            nc.gpsimd.dma_start(wt[:], t["wc2"][:, s * 24 * D:(s + 1) * 24 * D])
            wc.append(wt)

        out = t["out"]
        psum_c = ps_c.tile([P, D], FDT)

        # --- stages: T (PE transposes + DVE copy), M (matmuls) ---
        # Fine transposes are plain bf16 (6 per group); coarse transposes are
        # quad-packed fp32 views (4 fp8 chunks per [128,128] transpose) whose
        # interleave is undone in the DVE copy, so every matmul reads a
        # CONTIGUOUS lhsT (strided LDWEIGHTS costs ~40ns extra per matmul).
        def fine_T(g):
            tp = ps_tp.tile([P, KF], BDT, tag="tp")
            for j in range(KF // P):
                nc.tensor.transpose(
                    out=tp[:, j * P:(j + 1) * P],
                    in_=gfs[g][:, j * P:(j + 1) * P],
                    identity=identb[:],
                )
            lt = lt_f.tile([P, NQF * P], RDT, tag="ltf")
            nc.vector.tensor_copy(lt[:], tp[:].bitcast(RDT))
            return lt

        def fine_M(g, lt):
            psum = ps_f.tile([P, D], FDT, tag="psf")
            ltb = lt[:].bitcast(BDT)                   # [128, 768]
            for j in range(KF // P):
                nc.tensor.matmul(
                    out=psum[:],
                    lhsT=ltb[:, j * P:(j + 1) * P],
                    rhs=wf[:, j * D:(j + 1) * D],
                    start=(j == 0), stop=(j == KF // P - 1),
                )
            ob = ob_pool.tile([P, D], BDT, tag="ob")
            nc.vector.tensor_tensor(
                out=ob[:], in0=psum[:], in1=bias_f[:], op=mybir.AluOpType.add
            )
            b_img, hh = divmod(g, 2)
            row0 = b_img * (NF + NCO) + hh * P
            nc.scalar.dma_start(out[row0:row0 + P, :], ob[:])

        def coarse_T(tt):
            gc32 = gc[:].bitcast(RDT)                  # [128, 3072]
            tp = ps_tp.tile([P, 3 * P], RDT, tag="tp")
            for q in range(3):
                j = 3 * tt + q
                nc.tensor.transpose(
                    out=tp[:, q * P:(q + 1) * P],
                    in_=gc32[:, j * P:(j + 1) * P],
                    identity=ident[:],
                )
            lt = lt_c.tile([P, 3 * P], RDT, tag="ltc")
            # de-interleave the quad packing during the PSUM->SBUF copy:
            # tp8 col 512q + 4p + b  ->  lt8 col 512q + 128b + p
            tp8 = tp[:].bitcast(F8).rearrange("i (q p b) -> i q b p", q=3, p=P, b=4)
            lt8o = lt[:].bitcast(F8).rearrange("i (q b p) -> i q b p", q=3, b=4, p=P)
            nc.vector.tensor_copy(lt8o, tp8)
            return lt

        def coarse_M(tt, lt):
            lt8 = lt[:].bitcast(F8)                    # [128, 1536]
            for q in range(3):
                j = 3 * tt + q
                for b_par in range(4):
                    blk = 4 * j + b_par
                    nc.tensor.matmul(
                        out=psum_c[:],
                        lhsT=lt8[:, (4 * q + b_par) * P:(4 * q + b_par + 1) * P],
                        rhs=wc[blk // 24][:, (blk % 24) * D:(blk % 24 + 1) * D],
                        start=(blk == 0), stop=(blk == 4 * NQC - 1),
                    )
            if tt == NTC - 1:
                oc = ob_pool.tile([P, D], BDT, tag="ob")
                nc.vector.scalar_tensor_tensor(
                    out=oc[:], in0=psum_c[:], scalar=1.0 / S_W, in1=bias_c[:],
                    op0=mybir.AluOpType.mult, op1=mybir.AluOpType.add,
                )
                out3 = out[:].rearrange("(b r) d -> b r d", b=IPC)
                nc.scalar.dma_start(out3[:, NF:NF + NCO, :], oc[:])

        # --- emit with 1-stage software pipelining: T(s+1) before M(s) ---
        stages = [("f", g) for g in range(GF)] + [("c", tt) for tt in range(NTC)]
        prev = None
        for kind, i in stages:
            lt = fine_T(i) if kind == "f" else coarse_T(i)
            if prev is not None:
                pk, pi, plt = prev
                (fine_M if pk == "f" else coarse_M)(pi, plt)
            prev = (kind, i, lt)
        pk, pi, plt = prev
        (fine_M if pk == "f" else coarse_M)(pi, plt)


def build(reps: int = 1):
    nc = bacc.Bacc("TRN2", target_bir_lowering=False, debug=False)
    t = {
        "imgs16": nc.dram_tensor("imgs16", [IPC * IMG_F, 1], BDT, kind="ExternalInput"),
        "imgs8c": nc.dram_tensor("imgs8c", [IPC * IMG_C, 1], F8, kind="ExternalInput"),
        "wf2": nc.dram_tensor("wf2", [P, (KF // P) * D], BDT, kind="ExternalInput"),
        "wc2": nc.dram_tensor("wc2", [P, (KC // P) * D], F8, kind="ExternalInput"),
        "bias_f": nc.dram_tensor("bias_f", [P, D], FDT, kind="ExternalInput"),
        "bias_c": nc.dram_tensor("bias_c", [P, D], FDT, kind="ExternalInput"),
        "ident": nc.dram_tensor("ident", [P, P], RDT, kind="ExternalInput"),
        "identb": nc.dram_tensor("identb", [P, P], BDT, kind="ExternalInput"),
        "fidx": nc.dram_tensor("fidx", [P, GF], IDT, kind="ExternalInput"),
        "cidx": nc.dram_tensor("cidx", [P, 1], IDT, kind="ExternalInput"),
        "out": nc.dram_tensor("out", [IPC * (NF + NCO), D], BDT, kind="ExternalOutput"),
    }
    with tile.TileContext(nc) as tc:
        for _ in range(reps):
            _emit(nc, tc, t)
    nc.compile()
    return nc


def repack_fine(images):
    """[b, C, H, W] f32 -> sliding 16-row-block channel-last bf16 blob.

    blk[b, y, x, dy, c] = images[b, c, y+dy, x], y in [0, H-16]."""
    cl = np.ascontiguousarray(images.transpose(0, 2, 3, 1)).astype(BF16)
    sw = np.lib.stride_tricks.sliding_window_view(cl, FP, axis=1)  # [b,497,x,c,dy]
    return np.ascontiguousarray(sw.transpose(0, 1, 2, 4, 3))


def repack_coarse(images):
    """[b, C, H, W] f32 -> sliding 64-row-block channel-last e3m4 blob."""
    cl = np.ascontiguousarray(images.transpose(0, 2, 3, 1)).astype(E3M4)
    sw = np.lib.stride_tricks.sliding_window_view(cl, CP, axis=1)  # [b,449,x,c,dy]
    return np.ascontiguousarray(sw.transpose(0, 1, 2, 4, 3))


def host_indices(fine_xy, coarse_xy):
    """Element offsets into the per-core blobs (one per gather run)."""
    base_f = fine_xy[:, :, 1] * BLK_F + fine_xy[:, :, 0] * (FP * C) \
        + (np.arange(IPC) * IMG_F)[:, None]                        # [IPC, NF]
    fidx = base_f.reshape(GF, P).T                                 # [P, GF]
    base_c = coarse_xy[:, :, 1] * BLK_C + coarse_xy[:, :, 0] * (CP * C) \
        + (np.arange(IPC) * IMG_C)[:, None]                        # [IPC, NCO]
    cidx = base_c.reshape(P, 1)
    return (np.ascontiguousarray(fidx.astype(np.int32)),
            np.ascontiguousarray(cidx.astype(np.int32)))


def feat_perm(patch):
    """Gather order (dx, dy, c) -> original (c, dy, dx) column index."""
    dx, dy, c = np.meshgrid(
        np.arange(patch), np.arange(patch), np.arange(C), indexing="ij"
    )
    return (c * (patch * patch) + dy * patch + dx).reshape(-1)


def swizzle_w_interleave(wg, stride):
    """[K, D] gather-order weights -> [128, (K//128)*D], rows interleaved so
    block (j, b) holds rows (128*stride)*j + stride*i + b (i = partition)."""
    K = wg.shape[0]
    blocks = []
    for j in range(K // (P * stride)):
        for b in range(stride):
            blocks.append(wg[P * stride * j + stride * np.arange(P) + b])
    return np.ascontiguousarray(
        np.stack(blocks, axis=1).reshape(P, (K // P) * D)
    )


def make_in_maps(images, W_fine, b_fine, W_coarse, b_coarse, fine_xy, coarse_xy):
    images = np.asarray(images, dtype=np.float32)
    fine_xy = np.asarray(fine_xy, dtype=np.int64)
    coarse_xy = np.asarray(coarse_xy, dtype=np.int64)
    blob_f = repack_fine(images)
    blob_c = repack_coarse(images)
    wf2 = swizzle_w_interleave(
        np.asarray(W_fine, np.float32).T[feat_perm(FP)].astype(BF16), 1)
    wc2 = swizzle_w_interleave(
        (np.asarray(W_coarse, np.float32).T[feat_perm(CP)] * S_W).astype(E3M4), 4)
    bias_f = np.ascontiguousarray(
        np.repeat(np.asarray(b_fine, np.float32)[None, :], P, axis=0))
    bias_c = np.ascontiguousarray(
        np.repeat(np.asarray(b_coarse, np.float32)[None, :], P, axis=0))
    ident = np.eye(P, dtype=np.float32)
    identb = np.eye(P, dtype=BF16)
    in_maps = []
    for c in range(NCORES):
        sl = slice(c * IPC, (c + 1) * IPC)
        fidx, cidx = host_indices(fine_xy[sl], coarse_xy[sl])
        in_maps.append({
            "imgs16": blob_f[sl].reshape(IPC * IMG_F, 1),
            "imgs8c": blob_c[sl].reshape(IPC * IMG_C, 1),
            "wf2": wf2, "wc2": wc2,
            "bias_f": bias_f, "bias_c": bias_c, "ident": ident, "identb": identb,
            "fidx": fidx, "cidx": cidx,
        })
    return in_maps


_NC_CACHE = []


def _get_nc():
    if not _NC_CACHE:
        _NC_CACHE.append(build())
    return _NC_CACHE[0]


def run(inputs: dict, trace: bool = False):
    nc = _get_nc()
    in_maps = make_in_maps(**inputs)
    res = run_bass_kernel_spmd(nc, in_maps, list(range(NCORES)), trace=trace)
    outs = [
        np.asarray(res.results[c]["out"]).astype(np.float32).reshape(IPC, NF + NCO, D)
        for c in range(NCORES)
    ]
    return np.concatenate(outs, axis=0), res


def kernel(**inputs) -> np.ndarray:
    out, _ = run(inputs, trace=False)
    return out
